# revision 8
# baseline (speedup 1.0000x reference)
"""Trainium2 Bass kernel for nn_Attention (linear attention w/ cubed feature
map) — restructured v2.

Math (per batch b):
  q = relu(in1 @ W.T + pos) / s ;  k = relu(in2 + pos) / s ;  s = softplus(scale_p)
  qf = (||q||/||q^3||) * q^3    ;  kf = (||k||/||k^3||) * k^3
  kv[h] = (1/N) * kf_h.T @ v_h  (v = in2),  per head h (32-dim blocks)
  out = sigmoid(q_f @ blockdiag(kv)) * in1

v2 structural changes vs v1:
  - kv matmuls emitted as 2 x [128,128] per t-row-tile (c-halves) instead of
    8 x [32,32] per head: 128 instead of 512 PE instructions (PE sequencer
    was ~99% busy decoding in v1). Only the diagonal [32,32] blocks of the
    [128,128] products are extracted (4 strided DMAs per half) for the
    AllReduce; payload unchanged.
  - sigmoid's per-feature scale s[e] is folded into the kv block loads
    (tensor_mul with a host-shipped s-block constant instead of a copy), so
    the sigmoid is emitted j-fused as one [128,1024] op without a scale AP.
  - all q-side elementwise ops j-fused to [128,1024] (halved op count);
    per-op fixed costs on ACT are ~190-250ns.
  - tail matmuls drop the all-zero c-accumulate (1 matmul per (j,st)).
  - schedule: all mids -> all rats -> all tails, so the ACT table ping-pong
    between Sqrt and Sigmoid collapses to 2 LoadActFuncSet total (v1 had 9).
  - per-slice engine knobs tuned against the TimelineSim cost model: DVE
    stt 1x, TT 2x (bf16 SBUF), TSP 4x, f32/PSUM-sourced 1x; ACT ~0.83ns/elem
    + 185ns init + 187ns accum-read; Pool runs plain tensor_mul only
    (walrus rejects stt/tensor_scalar there) at ~2.1x DVE-TT cost.
  - first two k-side input DMAs are issued ahead of the const loads, and 6
    mid units of batches 0/1 are interleaved into the back half of window 1.

TimelineSim: 112,915 ns/core (baseline v1: 122,956). HW rel err vs the
fp64 reference: 2.9e-03 (v1: 1.4e-02 — v1's per-(b,h) PSUM start pattern
was dropping the first row-tile contribution of 7 of 8 heads per c-half).
"""

import numpy as np

B, N, D, H = 4, 16384, 256, 8
NCORES = 8
# hybrid sharding: each core owns ONE batch x HALF the sequence
NS = N // 2               # 8192 positions per core
PK = 4                    # k-side row-tiles per pack
NPK = (NS // 128) // PK   # packs per core = 16
ST = 512                  # q-side supertile rows
NST = NS // ST            # supertiles per core = 16

DEFAULT_CFG = dict(
    sim=False,       # single-core variant w/o collective (TimelineSim only)
    a1_f32=False,    # ship in1t as f32 + f32r proj (accuracy fallback)
    s3_act=30,       # of 64 s3 slices on ACT Square+accum (rest DVE stt)
    vp_act=10,       # of 64 vp scales on ACT (rest DVE TSP)
    k3_pool=0,       # of 16 k3 muls on Pool (keep off the k-chain)
    pf_act=1,        # route 2nd i2x prefetch via ACT DGE queue
    rbs_act=0,       # route last-half rbs bounce DMAs via ACT DGE queue
    a1_act=0,        # first N a1 loads via ACT DGE queue
    y_dve=0,         # of 16 y-relus on DVE (rest ACT; ACT is cheaper f32-in)
    q2_act=10,       # of 16 q2 squares on ACT
    q2_pool=1,       # of 16 q2 squares on Pool (rest DVE)
    w2_act=10,       # of 16 w2 squares on ACT
    w2_pool=1,       # of 16 w2 squares on Pool (rest DVE)
    q3_pool=1,       # of 16 q3 muls on Pool (rest DVE)
    wq_pool=2,       # of 16 wq muls on Pool (rest DVE)
    ot_pool=3,       # of 16 ot muls on Pool (rest DVE)
    kve_act=2,       # of 2 kv evac copies on ACT (rest DVE)
    kvl_pool=8,      # of 8 kv block loads on Pool (rest DVE)
    rbs_pb=0,        # of 4 batches using Pool partition_broadcast rbs
    dma_act=0,       # of 16 in2x loads issued via the ACT DGE queue
    early_mids=6,    # mid units (batches 0,1) pulled into window 1
    em_after=6,      # a-tile index after which early mids interleave
    # PSUM banks: kvps 1 + psq/px (2 banks x psq_bufs) + psel = 8
    psq_bufs=2, psel_bufs=2, kv_bufs=1,
    a1_bufs=16, y_bufs=16, q3_bufs=16,
)

_BUILT = {}


def build(cfg=None):
    cfg = dict(DEFAULT_CFG, **(cfg or {}))
    key = tuple(sorted(cfg.items()))
    if key in _BUILT:
        return _BUILT[key]

    import concourse.bacc as bacc
    import concourse.mybir as mybir
    import concourse.tile as tile

    f32 = mybir.dt.float32
    f32r = mybir.dt.float32r
    bf16 = mybir.dt.bfloat16
    a1dt = f32 if cfg["a1_f32"] else bf16
    AF = mybir.ActivationFunctionType
    ALU = mybir.AluOpType

    nc = bacc.Bacc("TRN2", target_bir_lowering=False, debug=False,
                   num_devices=(1 if cfg["sim"] else NCORES))

    in2x_d = nc.dram_tensor("in2x", [NS, 2, D], bf16, kind="ExternalInput")
    in1t_d = nc.dram_tensor("in1t", [D, NS], a1dt, kind="ExternalInput")
    post_d = nc.dram_tensor("post", [D, NS], bf16, kind="ExternalInput")
    wt_d = nc.dram_tensor("wt", [D, D], a1dt, kind="ExternalInput")
    sel_d = nc.dram_tensor("sel", [128, NST * 8], bf16, kind="ExternalInput")
    sblk_d = nc.dram_tensor("sblk", [128, 2, 32], bf16, kind="ExternalInput")
    iden_d = nc.dram_tensor("iden", [128, 128], bf16, kind="ExternalInput")
    outt_d = nc.dram_tensor("outt", [D, NS], bf16, kind="ExternalOutput")

    in2x_r = in2x_d.ap().rearrange("(pk t p) x f -> pk p x t f",
                                   pk=NPK, t=PK, p=128)
    in1t_r = in1t_d.ap().rearrange("(c p) r -> c p r", p=128)
    post_r = post_d.ap().rearrange("(c p) r -> c p r", p=128)
    wt_r = wt_d.ap().rearrange("(c p) e -> c p e", p=128)
    outt_r = outt_d.ap().rearrange("(c p) r -> c p r", p=128)

    with tile.TileContext(nc) as tc:
        with (
            tc.tile_pool(name="const", bufs=1) as constp,
            tc.tile_pool(name="dram", bufs=1, space="DRAM") as dram,
            tc.tile_pool(name="ka", bufs=3) as kap,      # A-side streams
            tc.tile_pool(name="kb", bufs=3) as kbp,      # A-side mids
            tc.tile_pool(name="ksml", bufs=4) as ksml,   # A-side per-row smalls
            tc.tile_pool(name="qa", bufs=2) as qap,      # B-side persisted
            tc.tile_pool(name="qb", bufs=3) as qbp,      # B-side transients
            tc.tile_pool(name="qs", bufs=2) as qsp,      # B-side smalls
            tc.tile_pool(name="kvps", bufs=cfg["kv_bufs"], space="PSUM") as kvpsp,
            tc.tile_pool(name="psq", bufs=cfg["psq_bufs"], space="PSUM") as psqp,
            tc.tile_pool(name="psel", bufs=cfg["psel_bufs"], space="PSUM") as pselp,
        ):
            # prefetch the first two k-side input tiles ahead of the
            # consts so the first k2 isn't stuck behind them on the SP queue
            i2x_pref = {}
            for pk in range(2):
                t = kap.tile([128, 2, PK, D], bf16, tag="i2x")
                eng = nc.scalar if (pk == 1 and cfg["pf_act"]) else nc.sync
                eng.dma_start(out=t[:], in_=in2x_r[pk])
                i2x_pref[pk] = t

            # ---- resident constants ----
            wt_sb = constp.tile([128, 2, D], a1dt, tag="wt")
            sel_sb = constp.tile([128, NST * 8], bf16, tag="sel")
            sblk_sb = constp.tile([128, 2, 32], bf16, tag="sblk")
            iden_sb = constp.tile([128, 128], bf16, tag="iden")
            for c in range(2):
                nc.sync.dma_start(out=wt_sb[:, c, :], in_=wt_r[c])
            nc.sync.dma_start(out=sblk_sb[:], in_=sblk_d.ap())
            nc.sync.dma_start(out=iden_sb[:], in_=iden_d.ap())

            def load_late_consts(step):
                if step == 0:
                    nc.sync.dma_start(out=sel_sb[:], in_=sel_d.ap())

            # kv result for this core's single batch; zeroed once, only the
            # block-diagonal slots are filled after the pair AllReduce.
            kvfall = constp.tile([128, 2, 128], bf16, tag="kvfall")
            nc.gpsimd.memset(kvfall[:], 0.0)

            cc_in = dram.tile([8, 32, 32], f32)
            cc_out = dram.tile([8, 32, 32], f32)
            rat_dr = dram.tile([NST, ST], bf16)

            # ---------------- emission helpers ----------------
            kv_ps_half = {}
            TOT = dict(s3=4 * PK * NPK // 4, vp=64, k3=16, y=16, q2=16,
                       w2=16, q3=16, wq=16, ot=16, kve=1, kvl=4, ai=16)
            cnt = {k: 0 for k in TOT}
            used = {k: [0, 0, 0] for k in TOT}

            def pick(key, n1, n2=0):
                """Largest-remainder spread of engine labels 1 (n1 slots) and
                2 (n2 slots) over TOT[key] slots; 0 = default engine."""
                i = cnt[key]
                cnt[key] += 1
                n = TOT[key]
                quotas = (n - n1 - n2, n1, n2)
                u = used[key]
                e = max(range(3),
                        key=lambda j: (quotas[j] * (i + 1) / n) - u[j])
                u[e] += 1
                return e

            def a_tile(pk):
                if pk in i2x_pref:
                    i2x = i2x_pref.pop(pk)
                else:
                    i2x = kap.tile([128, 2, PK, D], bf16, tag="i2x")
                    eng = nc.scalar if cnt["ai"] < cfg["dma_act"] else nc.sync
                    eng.dma_start(out=i2x[:], in_=in2x_r[pk])
                cnt["ai"] += 1
                tps = i2x[:, 0]
                i2s = i2x[:, 1]
                s1c = ksml.tile([128, PK], f32, tag="s1c")
                s3c = ksml.tile([128, PK], f32, tag="s3c")
                k2 = kbp.tile([128, PK, D], bf16, tag="k2")
                k3 = kbp.tile([128, PK, D], bf16, tag="k3")
                k6 = kbp.tile([128, PK, D], bf16, tag="k6")
                for t in range(PK):
                    nc.vector.scalar_tensor_tensor(
                        out=k2[:, t, :], in0=tps[:, t, :], scalar=0.0,
                        in1=tps[:, t, :], op0=ALU.max, op1=ALU.mult,
                        accum_out=s1c[:, t:t + 1])
                if pick("k3", cfg["k3_pool"]):
                    nc.gpsimd.tensor_mul(k3[:], k2[:], tps[:])
                else:
                    nc.vector.tensor_mul(k3[:], k2[:], tps[:])
                for t in range(PK):
                    if pick("s3", cfg["s3_act"]):
                        nc.scalar.activation(k6[:, t, :], k3[:, t, :],
                                             AF.Square,
                                             accum_out=s3c[:, t:t + 1])
                    else:
                        nc.vector.scalar_tensor_tensor(
                            out=k6[:, t, :], in0=k3[:, t, :], scalar=0.0,
                            in1=k3[:, t, :], op0=ALU.max, op1=ALU.mult,
                            accum_out=s3c[:, t:t + 1])
                rec = ksml.tile([128, PK], f32, tag="rec")
                nc.vector.reciprocal(rec[:], s3c[:])
                rr = ksml.tile([128, PK], f32, tag="rr")
                nc.vector.tensor_mul(rr[:], s1c[:], rec[:])
                rat = ksml.tile([128, PK], f32, tag="rat")
                nc.scalar.activation(rat[:], rr[:], AF.Sqrt)
                vp = kbp.tile([128, PK, D], bf16, tag="vp")
                for t in range(PK):
                    if pick("vp", cfg["vp_act"]):
                        nc.scalar.mul(vp[:, t, :], i2s[:, t, :],
                                      rat[:, t:t + 1])
                    else:
                        nc.vector.tensor_scalar_mul(
                            vp[:, t, :], i2s[:, t, :], rat[:, t:t + 1])
                # kv: one [128,128] matmul per (t, c); diagonal blocks only
                # are extracted later. Accumulates over (pk, t).
                # PSUM zero-region semantics: one start/stop per bank; the
                # c=1 area's first touch lands on pending-zero bytes.
                for t in range(PK):
                    for c in range(2):
                        nc.tensor.matmul(
                            kv_ps_half[0][:, c, :],
                            lhsT=k3[:, t, 128 * c:128 * (c + 1)],
                            rhs=vp[:, t, 128 * c:128 * (c + 1)],
                            start=(pk == 0 and t == 0 and c == 0),
                            stop=(pk == NPK - 1 and t == PK - 1 and c == 1))

            a1s = {}   # (b, st) -> a1 tile [128, 2, ST]
            ys = {}    # (b, st) -> y tile [128, 2, ST]

            b_pre_n = [0]

            def b_pre(st):
                a1 = qap.tile([128, 2, ST], a1dt, tag="a1",
                              name=f"a1_{st}", bufs=cfg["a1_bufs"])
                eng = (nc.scalar if b_pre_n[0] < cfg["a1_act"] else nc.sync)
                b_pre_n[0] += 1
                eng.dma_start(
                    out=a1[:],
                    in_=in1t_r[:, :, st * ST:(st + 1) * ST].rearrange(
                        "c p r -> p c r"))
                a1s[st] = a1
                # post streamed through a small ring instead of resident
                pch = kap.tile([128, 2, ST], bf16, tag="post", bufs=4)
                nc.sync.dma_start(
                    out=pch[:],
                    in_=post_r[:, :, st * ST:(st + 1) * ST].rearrange(
                        "c p r -> p c r"))
                psq = psqp.tile([128, 2, ST], f32, tag="psq")
                for j in range(2):
                    nc.tensor.matmul(
                        psq[:, j, :], lhsT=iden_sb[:],
                        rhs=pch[:, j, :],
                        start=True, stop=False)
                    for c in range(2):
                        nc.tensor.matmul(
                            psq[:, j, :],
                            lhsT=(wt_sb[:, c, j * 128:(j + 1) * 128].bitcast(f32r)
                                  if cfg["a1_f32"] else
                                  wt_sb[:, c, j * 128:(j + 1) * 128]),
                            rhs=(a1[:, c, :].bitcast(f32r) if cfg["a1_f32"]
                                 else a1[:, c, :]),
                            start=False, stop=(c == 1))
                y = qap.tile([128, 2, ST], bf16, tag="y",
                             name=f"y_{st}", bufs=cfg["y_bufs"])
                if pick("y", cfg["y_dve"]):
                    nc.vector.tensor_scalar_max(y[:], psq[:], 0.0)
                else:
                    nc.scalar.activation(y[:], psq[:], AF.Relu)
                ys[st] = y

            q3s = {}     # (b, st) -> q3 tile [128, 2, ST]
            psels = {}   # b -> live psel PSUM tile
            
            def b_mid(st):
                half, sh = st // 8, st % 8
                if sh == 0:
                    psel = pselp.tile([40, ST], f32, tag="psel",
                                      name=f"psel{half}")
                    psels[half] = psel
                ps14 = psels[half][0:8, :]
                ps34 = psels[half][32:40, :]
                y = ys[st]
                q2 = qbp.tile([128, 2, ST], bf16, tag="q2")
                e = pick("q2", cfg["q2_act"], cfg["q2_pool"])
                if e == 1:
                    nc.scalar.activation(q2[:], y[:], AF.Square)
                elif e == 2:
                    nc.gpsimd.tensor_mul(q2[:], y[:], y[:])
                else:
                    nc.vector.tensor_mul(q2[:], y[:], y[:])
                for j in range(2):
                    nc.tensor.matmul(
                        ps14,
                        lhsT=sel_sb[:, st * 8:(st + 1) * 8], rhs=q2[:, j, :],
                        start=(sh == 0 and j == 0),
                        stop=(sh == 7 and j == 1), skip_group_check=True)
                q3 = qap.tile([128, 2, ST], bf16, tag="q3",
                              name=f"q3_{st}", bufs=cfg["q3_bufs"])
                if pick("q3", cfg["q3_pool"]):
                    nc.gpsimd.tensor_mul(q3[:], q2[:], y[:])
                else:
                    nc.vector.tensor_mul(q3[:], q2[:], y[:])
                w2 = qbp.tile([128, 2, ST], bf16, tag="w2")
                e = pick("w2", cfg["w2_act"], cfg["w2_pool"])
                if e == 1:
                    nc.scalar.activation(w2[:], q3[:], AF.Square)
                elif e == 2:
                    nc.gpsimd.tensor_mul(w2[:], q3[:], q3[:])
                else:
                    nc.vector.tensor_mul(w2[:], q3[:], q3[:])
                for j in range(2):
                    nc.tensor.matmul(
                        ps34,
                        lhsT=sel_sb[:, st * 8:(st + 1) * 8], rhs=w2[:, j, :],
                        start=(sh == 0 and j == 0),
                        stop=(sh == 7 and j == 1), skip_group_check=True)
                q3s[st] = q3

            rbss = {}   # b -> rbs tile [128, NST, ST]

            def b_rat(half):
                # one fused rat chain per 8-supertile half: tails of the
                # first half unblock while mids of the second half run.
                ps = psels[half]
                rec4 = qsp.tile([8, ST], f32, tag="rec4", name=f"rec4_{half}",
                                bufs=2)
                nc.vector.reciprocal(rec4[:], ps[32:40, :])
                rr4 = qsp.tile([8, ST], f32, tag="rr4", name=f"rr4_{half}",
                               bufs=2)
                nc.vector.tensor_mul(rr4[:], ps[0:8, :], rec4[:])
                rat4 = qsp.tile([8, ST], bf16, tag="rat4",
                                name=f"rat4_{half}", bufs=2)
                nc.scalar.activation(rat4[:], rr4[:], AF.Sqrt,
                                     scale=1.0 / float(N) ** 2)
                sl = slice(8 * half, 8 * (half + 1))
                use_pb = 2 * half >= 4 - cfg["rbs_pb"]
                deng = nc.scalar if (half >= 2 - cfg["rbs_act"]) else nc.sync
                if not use_pb:
                    deng.dma_start(out=rat_dr[sl], in_=rat4[:])
                for g in (2 * half, 2 * half + 1):
                    rbs_b = qbp.tile([128, 4, ST], bf16, tag="rbs_b",
                                     name=f"rbs_{g}", bufs=4)
                    if use_pb:
                        # Pool broadcast: ready right after the sqrt, no
                        # DRAM round trip gating the late tails
                        gl = g - 2 * half
                        for s in range(4):
                            nc.gpsimd.partition_broadcast(
                                rbs_b[:, s, :],
                                rat4[4 * gl + s:4 * gl + s + 1, :])
                    else:
                        deng.dma_start(
                            out=rbs_b[:],
                            in_=rat_dr[4 * g:4 * (g + 1)].rearrange(
                                "(x s) r -> x s r", x=1).broadcast_to(
                                (128, 4, ST)))
                    rbss[g] = rbs_b

            def kv_evac():
                kv_sb = kbp.tile([128, 2, 128], f32, tag="kvsb",
                                 name="kvsb0")
                if pick("kve", cfg["kve_act"]):
                    nc.scalar.copy(kv_sb[:], kv_ps_half[0][:])
                else:
                    nc.vector.tensor_copy(kv_sb[:], kv_ps_half[0][:])
                cc_v = cc_in.rearrange("(c g) p e -> g p c e", c=2)
                for g in range(4):
                    nc.sync.dma_start(
                        out=cc_v[g],
                        in_=kv_sb[32 * g:32 * (g + 1), :,
                                  32 * g:32 * (g + 1)])

            def kv_load():
                stage = qbp.tile([128, 2, 32], f32, tag="kvstage",
                                 name="kvstage0")
                nc.sync.dma_start(
                    out=stage[:],
                    in_=cc_out.rearrange("(c g) q e -> (g q) c e", c=2))
                for g in range(4):
                    sl = slice(32 * g, 32 * (g + 1))
                    if pick("kvl", cfg["kvl_pool"]):
                        nc.gpsimd.tensor_mul(
                            kvfall[sl, :, sl], stage[sl], sblk_sb[sl])
                    else:
                        nc.vector.tensor_mul(
                            kvfall[sl, :, sl], stage[sl], sblk_sb[sl])

            def b_tail(st):
                q3 = q3s[st]
                rbs = rbss[st // 4][:, st % 4, :].rearrange(
                    "p (x r) -> p x r", x=1).broadcast_to((128, 2, ST))
                # wq written in-place into q3 (q3 is dead afterwards)
                if pick("wq", cfg["wq_pool"]):
                    nc.gpsimd.tensor_mul(q3[:], q3[:], rbs)
                else:
                    nc.vector.tensor_mul(q3[:], q3[:], rbs)
                px = psqp.tile([128, 2, ST], f32, tag="psq")
                for j in range(2):
                    nc.tensor.matmul(
                        px[:, j, :],
                        lhsT=kvfall[:, j, :],
                        rhs=q3[:, j, :],
                        start=True, stop=True)
                sg = qbp.tile([128, 2, ST], bf16, tag="sg")
                nc.scalar.activation(sg[:], px[:], AF.Sigmoid)
                ot = qbp.tile([128, 2, ST], bf16, tag="ot")
                a1 = a1s[st][:]
                if cfg["a1_f32"]:
                    a1 = a1.bitcast(f32)
                if pick("ot", cfg["ot_pool"]):
                    nc.gpsimd.tensor_mul(ot[:], sg[:], a1)
                else:
                    nc.vector.tensor_mul(ot[:], sg[:], a1)
                nc.sync.dma_start(
                    out=outt_r[:, :, st * ST:(st + 1) * ST].rearrange(
                        "c p r -> p c r"),
                    in_=ot[:])

            # ---------------- emission schedule ----------------
            # window 1: 16 A-tiles 1:1 with 16 B-pre units; early mids fill
            # the back half. One kv PSUM group, one pair-AllReduce at the end.
            kvh = kvpsp.tile([128, 2, 128], f32, tag="kv", name="kvps0")
            kv_ps_half[0] = kvh
            em = cfg["early_mids"]
            early = list(range(NST))[:em]
            early_it = iter(early)
            emitted_mids = set()

            def emit_early_mid():
                u = next(early_it, None)
                if u is None:
                    return
                b_mid(u)
                emitted_mids.add(u)

            for pk in range(NPK):
                a_tile(pk)
                if pk < NST // 4:
                    load_late_consts(pk)
                b_pre(pk)
                if pk > cfg["em_after"]:
                    emit_early_mid()
            kv_evac()
            if cfg["sim"]:
                nc.sync.dma_start(out=cc_out[:], in_=cc_in[:])
            else:
                nc.gpsimd.collective_compute(
                    "AllReduce", mybir.AluOpType.add,
                    replica_groups=[[2 * p, 2 * p + 1] for p in range(4)],
                    ins=[cc_in.opt()],
                    outs=[cc_out.opt()])

            # window 2: remaining mids, one fused rat chain, kv load, tails.
            for u in early_it:
                pass
            for st in range(NST):
                if st not in emitted_mids:
                    b_mid(st)
                if st == 7:
                    b_rat(0)
                    kv_load()
            b_rat(1)
            for st in range(NST):
                b_tail(st)

    nc.compile()
    _BUILT[key] = nc
    return nc


def _prep_inputs(input1, input2, conv_w, pos_enc, scale_p, cfg=None):
    import ml_dtypes
    cfg = dict(DEFAULT_CFG, **(cfg or {}))
    bf16 = ml_dtypes.bfloat16
    a1dt = np.float32 if cfg["a1_f32"] else bf16

    s64 = np.logaddexp(scale_p.reshape(-1).astype(np.float64), 0.0)  # [256]
    inv_s = (1.0 / s64).astype(np.float64)
    pos64 = pos_enc[0].astype(np.float64)                            # [N, 256]

    in2x_full = np.empty((B, N, 2, 256), dtype=bf16)
    in2x_full[:, :, 1, :] = (input2.astype(np.float64) * inv_s).astype(bf16)
    in2x_full[:, :, 0, :] = ((input2.astype(np.float64) + pos64)
                             * inv_s).astype(bf16)
    in1t_full = np.ascontiguousarray(
        input1.transpose(0, 2, 1)).astype(a1dt)                      # [B, D, N]
    post_full = np.ascontiguousarray((pos64 * inv_s).T).astype(bf16)  # [D, N]
    wtp = np.ascontiguousarray(
        (conv_w.astype(np.float64) * inv_s[:, None]).T).astype(a1dt)  # [d, e]

    iden = np.eye(128, dtype=bf16)
    sel = np.zeros((128, NST * 8), dtype=bf16)
    for st in range(NST):
        sel[:, st * 8 + st % 8] = 1
    # sblk[p, c, e] = s[128c + 32*(p//32) + e] — the sigmoid's per-feature
    # scale folded into the kv block loads.
    sblk = np.empty((128, 2, 32), dtype=bf16)
    s32 = s64.astype(np.float32)
    for c in range(2):
        for g in range(4):
            sblk[32 * g:32 * (g + 1), c, :] = s32[128 * c + 32 * g:
                                                  128 * c + 32 * (g + 1)]

    in_maps = []
    for core in range(NCORES):
        bc, hc = core // 2, core % 2
        sl = slice(hc * NS, (hc + 1) * NS)
        in_maps.append({
            "in2x": np.ascontiguousarray(in2x_full[bc, sl]),
            "in1t": np.ascontiguousarray(in1t_full[bc][:, sl]),
            "post": np.ascontiguousarray(post_full[:, sl]),
            "wt": wtp,
            "sel": sel,
            "sblk": sblk,
            "iden": iden,
        })
    return in_maps


def kernel(input1, input2, conv_w, pos_enc, scale_p, _cfg=None, _trace=False):
    from concourse import bass_utils
    nc = build(_cfg)
    in_maps = _prep_inputs(input1, input2, conv_w, pos_enc, scale_p, _cfg)
    res = bass_utils.run_bass_kernel_spmd(
        nc, in_maps, core_ids=list(range(NCORES)), trace=_trace)
    out = np.empty((B, N, D), np.float32)
    for core, r in enumerate(res.results):
        bc, hc = core // 2, core % 2
        out[bc, hc * NS:(hc + 1) * NS] = r["outt"].T.astype(np.float32)
    kernel._last_results = res
    return np.ascontiguousarray(out)
